# revision 1
# baseline (speedup 1.0000x reference)
"""Single-head attention (SEQ=8192, D_MODEL=2048, D_K=128) on 8 TRN2 NeuronCores.

Sequence-parallel: each core owns 1024 query rows. K^T and V are computed
per local sequence half and all-gathered in bf16; attention runs in S^T
layout ([key, query] tiles) over two query-half passes of 32 double-key-
block tiles each. The scalar engine's exp stream (64 x [128,1024] tiles,
~1.04us each) is the critical path; scores (bf16) and P@V (bf16) on the
tensor engine pipeline around it, and softmax denominators accumulate in
bf16 on the vector engine, partition-reduced by a ones-matmul at each
pass tail.

Scheduling (the tile scheduler assigns per-engine order greedily by
emission priority among ready instructions):
- the K0/Q0 projections + K0 gather/staging emit first: they gate exp-0;
- the other projections (V0, K1, V1, Q1) emit interleaved into the early
  phase-B tiles so their priorities sit just before their deadlines;
- x half-1 loads carry a ~18.5us wait so the FIFO DMA bus serves the
  gate-critical K-h0 staging chain first;
- low-priority filler matmuls keep the tensor engine's p-state clock at
  full speed (matmuls cost 2-4x more after the engine idles).
"""
import os

import numpy as np
import ml_dtypes

import concourse.bacc as bacc
import concourse.tile as tile
from concourse import mybir
from concourse.bass_utils import run_bass_kernel_spmd

N_CORES = 8
SEQ = 8192
DM = 2048
DK = 128
SL = SEQ // N_CORES          # 1024 local rows
NMC = DM // 128              # 16 contraction chunks for projections
XG = 4                       # x chunks per DMA group
NG = NMC // XG               # 4 groups per sequence half
SCALE = float(np.sqrt(DK))

F32 = mybir.dt.float32
BF16 = mybir.dt.bfloat16
ID = mybir.ActivationFunctionType.Identity
EXP = mybir.ActivationFunctionType.Exp


def _build():
    nc = bacc.Bacc(
        "TRN2",
        target_bir_lowering=False,
        debug=False,
        num_devices=N_CORES,
    )

    # host-prepacked partition-major layouts for full-bandwidth loads
    xP = nc.dram_tensor("xP", [128, 4, NMC, 256], BF16,
                    kind="ExternalInput")
    wqP = nc.dram_tensor("wqP", [128, NMC, DK], BF16, kind="ExternalInput")
    wkP = nc.dram_tensor("wkP", [128, NMC, DK], BF16, kind="ExternalInput")
    wvP = nc.dram_tensor("wvP", [128, NMC, DK], BF16, kind="ExternalInput")
    # [:, 0:128] ones, [:, 128:131] bq/bk/bv biases
    consts_d = nc.dram_tensor("consts_d", [128, 132], BF16,
                              kind="ExternalInput")
    out = nc.dram_tensor("out", [DK, SL], F32, kind="ExternalOutput")

    skip_cc = os.environ.get("KCC", "") == "skip"
    groups = [list(range(N_CORES))]

    with tile.TileContext(nc) as tc:
        with (
            tc.tile_pool(name="const", bufs=1) as const_pool,
            tc.tile_pool(name="w", bufs=1) as w_pool,
            tc.tile_pool(name="proj", bufs=1) as proj_pool,
            tc.tile_pool(name="kv", bufs=1) as kv_pool,
            tc.tile_pool(name="pt", bufs=16) as pt_pool,
            tc.tile_pool(name="fin", bufs=1) as fin_pool,
            tc.tile_pool(name="dram", bufs=1, space="DRAM") as dram_pool,
            tc.tile_pool(name="ps_a", bufs=2, space="PSUM") as ps_a,
            tc.tile_pool(name="ps_st", bufs=2, space="PSUM") as ps_st,
            tc.tile_pool(name="ps_o", bufs=1, space="PSUM") as ps_o,
            tc.tile_pool(name="ps_w", bufs=1, space="PSUM") as ps_w,
        ):
            # ---- SBUF tiles ----
            consts = const_pool.tile([128, 132], BF16)
            ones_r = consts[:, 0:128]
            bq_sb = consts[:, 128:129]
            bk_sb = consts[:, 129:130]
            bv_sb = consts[:, 130:131]

            wk_t = w_pool.tile([128, NMC, DK], BF16)
            wq_t = w_pool.tile([128, NMC, DK], BF16)
            wv_t = w_pool.tile([128, NMC, DK], BF16)
            wk_sb = [wk_t[:, i, :] for i in range(NMC)]
            wq_sb = [wq_t[:, i, :] for i in range(NMC)]
            wv_sb = [wv_t[:, i, :] for i in range(NMC)]
            # x quarters: [128, chunk, 256 seq]; quarter q = local seq
            # columns q*256..(q+1)*256 (all d_model chunks)
            x_q = [w_pool.tile([128, NMC, 256], BF16, name=f"xq{q}")
                   for q in range(4)]

            # local K^T quarters / Q^T / V^T halves (bf16, biased)
            kq_loc = [proj_pool.tile([128, 256], BF16, name=f"kql{q}")
                      for q in range(4)]
            qt_sb = [proj_pool.tile([128, 512], BF16, name=f"qtl{h}")
                     for h in range(2)]
            vt_h = [proj_pool.tile([128, 512], BF16, name=f"vth{h}")
                    for h in range(2)]
            vsb_h = [proj_pool.tile([128, 4, 128], BF16, name=f"vs{h}")
                     for h in range(2)]

            # gathered K^T (per source quarter) / V, split per staging
            # writer so readers don't serialize on unrelated DMAs
            kth_q = [[kv_pool.tile([128, nb, 256], BF16, name=f"kq{q}{i}")
                      for i, nb in enumerate((2, 6))] for q in range(4)]
            v_t = [kv_pool.tile([128, N_CORES, 128], BF16, name=f"van{ht}")
                   for ht in range(8)]

            # DRAM staging + gathered buffers
            ktd_q = [dram_pool.tile([128, 256], BF16, name=f"ktd{q}")
                     for q in range(4)]
            vd_h = [dram_pool.tile([512, DK], BF16, name=f"vd{h}")
                    for h in range(2)]
            ktg_q = [dram_pool.tile([N_CORES, 128, 256], BF16,
                                    addr_space="Shared", name=f"ktg{q}")
                     for q in range(4)]
            vg_h = [dram_pool.tile([N_CORES, 512, DK], BF16,
                                   addr_space="Shared", name=f"vg{h}")
                    for h in range(2)]

            # ---- input DMA stream (SP queue); x-h0 g0 first so the K
            # projection starts ASAP ----
            nc.sync.dma_start(wk_t[:], wkP[:])
            nc.sync.dma_start(x_q[0][:], xP[:, 0, :, :])
            nc.sync.dma_start(consts[:], consts_d[:])
            nc.sync.dma_start(wq_t[:], wqP[:])
            nc.sync.dma_start(x_q[1][:], xP[:, 1, :, :])
            nc.sync.dma_start(wv_t[:], wvP[:])

            # ---- projection helpers ----
            def projq_matmuls(ps, w_sb, q, col, lo, hi):
                # accumulate one seq-quarter of a projection into
                # ps[:, col*256:(col+1)*256]
                cs_ = slice(col * 256, (col + 1) * 256)
                for i in range(lo, hi):
                    nc.tensor.matmul(ps[:, cs_], w_sb[i], x_q[q][:, i, :],
                                     start=(i == 0), stop=(i == NMC - 1))

            def proj2_matmuls(psA, psB, w_sb, h, lo, hi):
                # one PSUM bank per accumulation group (start=True zeroes
                # a full 2KB region)
                for i in range(lo, hi):
                    projq_matmuls(psA, w_sb, 2 * h, 0, i, i + 1)
                    projq_matmuls(psB, w_sb, 2 * h + 1, 0, i, i + 1)

            def k_finish(ps, q, eng):
                nc.scalar.activation(kq_loc[q][:], ps[:, 0:256], ID,
                                     bias=bk_sb[:])
                eng.dma_start(ktd_q[q][:], kq_loc[q][:])

            def k_gather(q, eng):
                if not skip_cc:
                    nc.gpsimd.collective_compute(
                        "AllGather", mybir.AluOpType.bypass,
                        replica_groups=groups,
                        ins=[ktd_q[q].opt()], outs=[ktg_q[q].opt()],
                    )
                eng.dma_start(
                    kth_q[q][0][:],
                    ktg_q[q][0:2].rearrange("b p c -> p b c"))
                eng.dma_start(
                    kth_q[q][1][:],
                    ktg_q[q][2:8].rearrange("b p c -> p b c"))

            def q_finish(psA, psB, h):
                nc.scalar.activation(qt_sb[h][:, 0:256], psA[:, 0:256], ID,
                                     bias=bq_sb[:])
                nc.scalar.activation(qt_sb[h][:, 256:512], psB[:, 0:256], ID,
                                     bias=bq_sb[:])

            def v_finish(psA, psB, h):
                nc.scalar.activation(vt_h[h][:, 0:256], psA[:, 0:256], ID,
                                     bias=bv_sb[:])
                nc.scalar.activation(vt_h[h][:, 256:512], psB[:, 0:256], ID,
                                     bias=bv_sb[:])
                for t in range(4):
                    nc.sync.dma_start(
                        vsb_h[h][:, t, :],
                        vt_h[h][:, t * 128:(t + 1) * 128], transpose=True)
                nc.sync.dma_start(
                    vd_h[h].rearrange("(t p) d -> p t d", p=128),
                    vsb_h[h][:])

            def v_gather(h):
                if not skip_cc:
                    nc.gpsimd.collective_compute(
                        "AllGather", mybir.AluOpType.bypass,
                        replica_groups=groups,
                        ins=[vd_h[h].opt()], outs=[vg_h[h].opt()],
                    )
                for t in range(4):
                    nc.gpsimd.dma_start(
                        v_t[h * 4 + t][:],
                        vg_h[h][:, t * 128:(t + 1) * 128, :].rearrange(
                            "b p d -> p b d"))

            # ---- phase A prefix: K-q0 -> gather (gates exp-0), Q-h0,
            # then K-q1 -> gather (gates tile 8) ----
            kt_psA = ps_a.tile([128, 512], F32, tag="pa")
            projq_matmuls(kt_psA, wk_sb, 0, 0, 0, NMC)
            k_finish(kt_psA, 0, nc.scalar)

            qt_psA = ps_a.tile([128, 512], F32, tag="pa")
            qt_psB = ps_a.tile([128, 512], F32, tag="pa")
            proj2_matmuls(qt_psA, qt_psB, wq_sb, 0, 0, NMC)
            q_finish(qt_psA, qt_psB, 0)
            k_gather(0, nc.scalar)

            kt_psB = ps_a.tile([128, 512], F32, tag="pa")
            projq_matmuls(kt_psB, wk_sb, 1, 0, 0, NMC)
            k_finish(kt_psB, 1, nc.scalar)
            k_gather(1, nc.scalar)

            vt_psA = ps_a.tile([128, 512], F32, tag="pa")
            vt_psB = ps_a.tile([128, 512], F32, tag="pa")
            kt_psC = ps_a.tile([128, 512], F32, tag="pa")
            kt_psD = ps_a.tile([128, 512], F32, tag="pa")
            vt_psC = ps_a.tile([128, 512], F32, tag="pa")
            vt_psD = ps_a.tile([128, 512], F32, tag="pa")
            qt_psC = ps_a.tile([128, 512], F32, tag="pa")
            qt_psD = ps_a.tile([128, 512], F32, tag="pa")

            def emit_xh1():
                # hold x-h1 until the K-h0 staging chains have taken their
                # FIFO bus slots (they gate the exp stream)
                with tc.tile_wait_until(0.013):
                    nc.sync.dma_start(x_q[2][:], xP[:, 2, :, :])
                    nc.sync.dma_start(x_q[3][:], xP[:, 3, :, :])

            # extras interleaved into pass-0 tiles at deadline-relative
            # priorities (tile j of pass 0 runs ~ exp-0 + j us)
            extras = {
                0: [lambda: proj2_matmuls(vt_psA, vt_psB, wv_sb, 0, 0, 4)],
                1: [lambda: proj2_matmuls(vt_psA, vt_psB, wv_sb, 0, 4, 9),
                    emit_xh1],
                2: [lambda: proj2_matmuls(vt_psA, vt_psB, wv_sb, 0, 9, 16)],
                3: [lambda: v_finish(vt_psA, vt_psB, 0),
                    lambda: v_gather(0),
                    lambda: projq_matmuls(kt_psC, wk_sb, 2, 0, 0, 8)],
                4: [lambda: projq_matmuls(kt_psC, wk_sb, 2, 0, 8, NMC)],
                5: [lambda: k_finish(kt_psC, 2, nc.sync),
                    lambda: k_gather(2, nc.gpsimd)],
                6: [lambda: projq_matmuls(kt_psD, wk_sb, 3, 0, 0, 8)],
                7: [lambda: projq_matmuls(kt_psD, wk_sb, 3, 0, 8, NMC)],
                8: [lambda: k_finish(kt_psD, 3, nc.sync),
                    lambda: k_gather(3, nc.gpsimd)],
                9: [lambda: proj2_matmuls(vt_psC, vt_psD, wv_sb, 1, 0, 2)],
                10: [lambda: proj2_matmuls(vt_psC, vt_psD, wv_sb, 1, 2, 4)],
                11: [lambda: proj2_matmuls(vt_psC, vt_psD, wv_sb, 1, 4, 7)],
                12: [lambda: proj2_matmuls(vt_psC, vt_psD, wv_sb, 1, 7, 10)],
                13: [lambda: proj2_matmuls(vt_psC, vt_psD, wv_sb, 1, 10, 13)],
                14: [lambda: proj2_matmuls(vt_psC, vt_psD, wv_sb, 1, 13, 16)],
                15: [lambda: v_finish(vt_psC, vt_psD, 1),
                    lambda: v_gather(1)],
                16: [lambda: proj2_matmuls(qt_psC, qt_psD, wq_sb, 1, 0, 2)],
                17: [lambda: proj2_matmuls(qt_psC, qt_psD, wq_sb, 1, 2, 4)],
                18: [lambda: proj2_matmuls(qt_psC, qt_psD, wq_sb, 1, 4, 6)],
                19: [lambda: proj2_matmuls(qt_psC, qt_psD, wq_sb, 1, 6, 8)],
                20: [lambda: proj2_matmuls(qt_psC, qt_psD, wq_sb, 1, 8, 10)],
                21: [lambda: proj2_matmuls(qt_psC, qt_psD, wq_sb, 1, 10, 12)],
                22: [lambda: proj2_matmuls(qt_psC, qt_psD, wq_sb, 1, 12, 14)],
                23: [lambda: proj2_matmuls(qt_psC, qt_psD, wq_sb, 1, 14, 16),
                     lambda: q_finish(qt_psC, qt_psD, 1)],
            }

            # ---- phase B: two query-half passes over 32 double-key-block
            # tiles; exp on ACT is the critical stream ----
            # quarter-major: tile (h,b,tp) needs exactly K-quarter 2h+tp
            js = [(q // 2, b, q % 2) for q in range(4)
                  for b in range(N_CORES)]
            NT = len(js)

            for u in range(2):
                lag = 8 if u == 0 else 1
                o_ps = ps_o.tile([128, 512], F32, tag="o")
                acc = fin_pool.tile([128, 2, 512], BF16, name=f"acc{u}")
                us = slice(u * 512, (u + 1) * 512)
                pts = {}
                n_acc = 0

                def emit_pv(j):
                    h, b, tp = js[j]
                    pt = pts[j]
                    for s in range(2):
                        nc.tensor.matmul(
                            o_ps[:], v_t[h * 4 + 2 * tp + s][:, b, :],
                            pt[:, s, :],
                            start=(j == 0 and s == 0),
                            stop=(j == NT - 1 and s == 1))

                for j in range(NT):
                    h, b, tp = js[j]
                    st = ps_st.tile([128, 2, 512], F32, tag="st")
                    kht = kth_q[2 * h + tp][0 if b < 2 else 1]
                    bi = b if b < 2 else b - 2
                    for s in range(2):
                        nc.tensor.matmul(
                            st[:, s, :],
                            kht[:, bi, s * 128:(s + 1) * 128],
                            qt_sb[u][:],
                            start=True, stop=True)
                    pt = pt_pool.tile([128, 2, 512], BF16, tag="pt")
                    nc.scalar.activation(pt[:], st[:], EXP, scale=1.0 / SCALE)
                    pts[j] = pt

                    if u == 0:
                        for fn in extras.pop(j, []):
                            fn()

                    if j >= lag:
                        emit_pv(j - lag)

                    # bf16 accumulation of exp sums on DVE (tile NT-1 is
                    # reduced directly by the ones-matmul below)
                    if j < NT - 1:
                        if n_acc == 0:
                            nc.vector.tensor_copy(acc[:], pt[:])
                        else:
                            nc.vector.tensor_add(acc[:], acc[:], pt[:])
                        n_acc += 1

                # pass-0's tail (leftover PVs, denominator,
                # reciprocal, output) must not block pass-1's
                # scores in the in-order PE queue: deprioritize
                with tc.high_priority(offset=(-50000 if u == 0 else 0)):
                    for j in range(NT - lag, NT):
                        emit_pv(j)

                    # partition-reduce denominators: ones^T @ {acc, pt_last}.
                    # cs lives in the pa ring (all projection tiles are long
                    # released); using the st ring here would couple pass-1's
                    # scores pipeline to this pass's reciprocal.
                    cs_t = ps_a.tile([128, 512], F32, tag="pa")
                    cs = cs_t[:]
                    srcs = [acc[:, 0, :], acc[:, 1, :],
                            pts[NT - 1][:, 0, :], pts[NT - 1][:, 1, :]]
                    for i, sap in enumerate(srcs):
                        nc.tensor.matmul(cs[:], ones_r[:], sap,
                                         start=(i == 0), stop=(i == len(srcs) - 1))
                    rcs = fin_pool.tile([128, 512], F32, name=f"rcs{u}")
                    # halves pipelined: reciprocal/multiply overlap the out DMA
                    for e in range(2):
                        es = slice(e * 256, (e + 1) * 256)
                        nc.vector.reciprocal(rcs[:, es], cs[:, es])
                        nc.vector.tensor_mul(rcs[:, es], o_ps[:, es], rcs[:, es])
                        nc.sync.dma_start(out[:, u * 512 + e * 256:
                                              u * 512 + (e + 1) * 256],
                                          rcs[:, es])
                pts.clear()

            # ---- PE warmers: lowest scheduler preference, run only when
            # nothing else is ready. Keeping the PE queue non-empty holds
            # the tensor engine at its full p-state clock. ----
            with tc.high_priority(offset=-200000):
                warm = ps_w.tile([128, 512], F32, tag="w")
                wsrc = x_q[0][:, 0:2, :]
                for i in range(36):
                    nc.tensor.matmul(
                        warm[:], ones_r[:], wsrc,
                        start=True, stop=True, skip_group_check=True)
                for i in range(15):
                    nc.tensor.matmul(
                        warm[:, 0:128], ones_r[:], ones_r[:],
                        start=True, stop=True, skip_group_check=True)

    nc.compile()
    return nc


_NC_CACHE = {}


def _get_nc():
    key = os.environ.get("KCC", "")
    if key not in _NC_CACHE:
        _NC_CACHE[key] = _build()
    return _NC_CACHE[key]


def _run(inputs, trace=False, **spmd_kwargs):
    BF = ml_dtypes.bfloat16
    x = np.asarray(inputs["x"], dtype=np.float32)
    Wq = np.asarray(inputs["Wq"], dtype=np.float32)
    Wk = np.asarray(inputs["Wk"], dtype=np.float32)
    Wv = np.asarray(inputs["Wv"], dtype=np.float32)
    bq = np.asarray(inputs["bq"], dtype=np.float32)
    bk = np.asarray(inputs["bk"], dtype=np.float32)
    bv = np.asarray(inputs["bv"], dtype=np.float32)

    def prepack(wT):
        # [DM, DK] -> [128, NMC, DK]: partition p, chunk c holds row c*128+p
        return np.ascontiguousarray(
            wT.reshape(NMC, 128, -1).transpose(1, 0, 2)).astype(BF)

    consts = np.zeros((128, 132), dtype=np.float32)
    consts[:, 0:128] = 1.0
    consts[:, 128] = bq
    consts[:, 129] = bk
    consts[:, 130] = bv
    shared = {
        "wqP": prepack(Wq.T),
        "wkP": prepack(Wk.T),
        "wvP": prepack(Wv.T),
        "consts_d": consts.astype(BF),
    }
    in_maps = []
    for c in range(N_CORES):
        xT_c = np.ascontiguousarray(x[c * SL:(c + 1) * SL].T)
        # [128 part, 4 seq-quarter, 16 chunk, 256]: row c*128+p, col q*256+s
        xq = np.ascontiguousarray(
            xT_c.reshape(NMC, 128, 4, 256).transpose(1, 2, 0, 3)).astype(BF)
        in_maps.append({"xP": xq, **shared})

    nc = _get_nc()
    res = run_bass_kernel_spmd(
        nc, in_maps, core_ids=list(range(N_CORES)), trace=trace, **spmd_kwargs)
    full = np.concatenate(
        [np.ascontiguousarray(res.results[c]["out"].T)
         for c in range(N_CORES)], axis=0)
    return full, res


def kernel(**inputs):
    out, _ = _run(inputs)
    return out

